# revision 2
# baseline (speedup 1.0000x reference)
"""Trainium2 Bass kernel for MllamaTextSelfAttention (B=1, S=2048, HID=4096,
32 Q heads / 8 KV heads, HD=128, RoPE, causal mask, GQA).

Sharding: tensor-parallel over heads across 8 NeuronCores. Core c computes
Q heads [4c, 4c+4) and KV head c, plus the matching slice of the output
projection; the 8 partial outputs are summed on the host.

Dataflow (per core; fp32r matmuls except bf16 weights/h; free dim 512):
  phase 1 - batched bf16 weight DMAs in consumption order; ht chunks stream
    with deep buffering (issues on SP + Pool-SWDGE, or SP+Act in the For_i
    timing build); W-stationary QKV projections; RoPE as a rotate-matmul on
    the PE (psr = P_rot @ x) plus 3 DVE ops; V transposed via the PE.
  phase 2+3 - per (qs, h): S^T chunks [128,1024] in PSUM, causal diag mask
    accumulated on the PE via an identity-stationary matmul, exp on Act
    straight from PSUM; denominator = DVE chunk-sum + a single
    ones[128,128]-matmul pair (k-partition reduce + broadcast);
    od = pso * recip(psd) in one DVE op; the output projection is
    interleaved one stripe behind attention as PE-dense filler.
"""

import math
import os
import sys

for _p in (
    "/opt/trn_rl_repo",
    "/root/.axon_site",
    "/root/.axon_site/_ro/trn_rl_repo",
    "/root/.axon_site/_ro/pypackages",
):
    if os.path.isdir(_p) and _p not in sys.path:
        sys.path.append(_p)

import numpy as np
from contextlib import ExitStack

import concourse.bass as bass
import concourse.tile as tile
from concourse import mybir
from concourse.bass_utils import run_bass_kernel_spmd
from concourse.masks import make_identity

F32 = mybir.dt.float32
FR = mybir.dt.float32r
BF16 = mybir.dt.bfloat16
ACTF = mybir.ActivationFunctionType

B, S, HID = 1, 2048, 4096
NH, NKV, HD = 32, 8, 128
NCORES = 8
QH = NH // NCORES          # 4 q heads per core
SS = 512                   # stripe / matmul free dim
NQS = S // SS              # 4 stripes
NKT = S // 128             # 16 k tiles
KH = HID // 128            # 32 hidden-dim chunks
NEG = -1e9


def _split_multi_waits(nc: bass.Bass):
    """Walrus in this container encodes at most ONE sync-wait command per
    instruction. Hoist extra waits onto injected same-engine NoOps placed
    immediately before the instruction; engines are in-order so the
    semantics are unchanged."""
    n = 0
    for fn in nc.m.functions:
        for bb in fn.blocks:
            out = []
            for inst in bb.instructions:
                si = inst.sync_info
                if si is not None and si.on_wait and len(si.on_wait) > 1:
                    waits = list(si.on_wait)
                    for w in waits[:-1]:
                        n += 1
                        nop = mybir.InstNoOp(name=f"I-swait-{n}", ins=[], outs=[])
                        nop.engine = inst.engine
                        nop.sync_info = mybir.SyncInfo(on_wait=[w], on_update=[])
                        out.append(nop)
                    si.on_wait = [waits[-1]]
                out.append(inst)
            bb.instructions[:] = out
    return nc


_BUILD_CACHE = {}


def _build(causal: bool, split_waits: bool = True, loop_n=None, phases="all") -> bass.Bass:
    key = (causal, split_waits, loop_n, phases)
    if key in _BUILD_CACHE:
        return _BUILD_CACHE[key]

    nc = bass.Bass()
    hT = nc.dram_tensor("hT", [HID, S], BF16, kind="ExternalInput")
    wqL = nc.dram_tensor("wqL", [128, KH * 512], BF16, kind="ExternalInput")
    wkL = nc.dram_tensor("wkL", [128, KH * 128], BF16, kind="ExternalInput")
    wvL = nc.dram_tensor("wvL", [128, KH * 128], BF16, kind="ExternalInput")
    woL = nc.dram_tensor("woL", [128, QH * HID], BF16, kind="ExternalInput")
    cosT = nc.dram_tensor("cosT", [128, S], F32, kind="ExternalInput")
    sinT = nc.dram_tensor("sinT", [128, S], F32, kind="ExternalInput")
    protD = nc.dram_tensor("protD", [128, 128], F32, kind="ExternalInput")
    maskd = maskT = None
    if causal:
        maskd = nc.dram_tensor("maskd", [128, 4 * SS], F32, kind="ExternalInput")
    else:
        maskT = nc.dram_tensor("maskT", [S, S], F32, kind="ExternalInput")
    y = nc.dram_tensor("y", [S, HID], F32, kind="ExternalOutput")

    with tile.TileContext(nc) as tc, ExitStack() as ctx:
        # SWDGE (gpsimd) DMA inside a For_i fails this walrus' codegen; the
        # timing variants route the Pool-issued DMAs through SP instead.
        gp = nc.sync if loop_n is not None else nc.gpsimd

        if loop_n is not None and phases != "split":
            ctx.enter_context(tc.For_i(0, loop_n, 1))

        outer = ctx.enter_context(tc.tile_pool(name="outer", bufs=1))
        qT = outer.tile([128, QH * S], FR)      # [d, h*s] rope'd Q
        kT = outer.tile([128, S], FR)           # [d, s]   rope'd K
        v_sb = outer.tile([128, S], FR)         # [s-within-tile, t*d]
        ot = outer.tile([128, QH * S], BF16)    # [d, h*s] normalized O^T
        wo_all = outer.tile([128, QH * HID], BF16)
        ones_f32 = outer.tile([128, 128], F32)
        ones_sb = outer.tile([128, 128], FR)
        cos_sb = outer.tile([128, S], F32)
        sin_sb = outer.tile([128, S], F32)
        prot_sb = outer.tile([128, 128], FR)
        id_fr = outer.tile([128, 128], FR)
        md_sb = None
        if causal:
            md_sb = outer.tile([128, 4 * SS], FR, name="md_sb")

    # ---------------- phase 1: QKV + RoPE + V transpose -----------------
        def emit_phase1():
          with (
            tc.tile_pool(name="wqkv", bufs=1) as wp,
            tc.tile_pool(name="hstream", bufs=8) as hp,
            tc.tile_pool(name="rsc", bufs=1) as rsc,
            tc.tile_pool(name="ps1", bufs=1, space="PSUM") as pp1,
            tc.tile_pool(name="psr", bufs=1, space="PSUM") as ppr,
            tc.tile_pool(name="pst", bufs=1, space="PSUM") as ppt,
          ):
            wq_all = wp.tile([128, KH * 512], BF16)
            wk_all = wp.tile([128, KH * 128], BF16)
            wv_all = wp.tile([128, KH * 128], BF16)
            id_sb = wp.tile([128, 128], F32)
            vT = wp.tile([128, SS], F32)

            # batched weight loads, issued in chunk-consumption order so the
            # stripe-0 matmuls stream as parts arrive
            def wq_part(p):  # 4 chunks each
                gp.dma_start(wq_all[:, p * 2048 : (p + 1) * 2048],
                             wqL[:, p * 2048 : (p + 1) * 2048])
            def wkv_part(p):  # 8 chunks each
                gp.dma_start(wk_all[:, p * 1024 : (p + 1) * 1024],
                             wkL[:, p * 1024 : (p + 1) * 1024])
                gp.dma_start(wv_all[:, p * 1024 : (p + 1) * 1024],
                             wvL[:, p * 1024 : (p + 1) * 1024])
            wq_part(0); wkv_part(0); wq_part(1)
            wkv_part(1); wq_part(2); wq_part(3)
            wkv_part(2); wq_part(4); wq_part(5)
            wkv_part(3); wq_part(6); wq_part(7)
            gp.dma_start(prot_sb[:], protD[:, :].bitcast(FR))
            # after the weight parts: lands ~40us in, before stripe-0 rope
            gp.dma_start(cos_sb[:], cosT[:, :])
            gp.dma_start(sin_sb[:], sinT[:, :])
            make_identity(nc, id_sb[:])
            nc.vector.tensor_copy(id_fr[:], id_sb[:])
            nc.vector.memset(ones_f32[:], 1.0)
            nc.vector.tensor_copy(ones_sb[:], ones_f32[:])

            # Pool is idle after the preamble in SWDGE mode; in the For_i
            # timing build SWDGE is unavailable so Act helps SP instead
            dma_engs = [nc.sync, nc.gpsimd] if loop_n is None else [nc.sync, nc.scalar]
            for n in range(NQS):
                if n == 2:
                    nc.sync.dma_start(wo_all[:], woL[:, :])
                    if causal:
                        nc.sync.dma_start(md_sb[:], maskd[:, :].bitcast(FR))
                psq = [
                    pp1.tile([128, SS], F32, name=f"psq{m}", tag=f"psq{m}")
                    for m in range(QH)
                ]
                psk = pp1.tile([128, SS], F32, tag="psk")
                psv = pp1.tile([128, SS], F32, tag="psv")
                for k in range(KH):
                    ht = hp.tile([128, SS], BF16, tag="ht")
                    dma_engs[k % 2].dma_start(
                        ht[:],
                        hT[k * 128 : (k + 1) * 128, n * SS : (n + 1) * SS],
                    )
                    st, sp = (k == 0), (k == KH - 1)
                    for m in range(QH):
                        nc.tensor.matmul(
                            psq[m][:],
                            wq_all[:, k * 512 + m * 128 : k * 512 + (m + 1) * 128],
                            ht[:],
                            start=st,
                            stop=sp,
                        )
                    nc.tensor.matmul(
                        psk[:], wk_all[:, k * 128 : (k + 1) * 128], ht[:],
                        start=st, stop=sp,
                    )
                    nc.tensor.matmul(
                        psv[:], wv_all[:, k * 128 : (k + 1) * 128], ht[:],
                        start=st, stop=sp,
                    )
                # PSUM -> SBUF copies, split across Act and DVE so the next
                # stripe's start matmuls unblock quickly
                nc.scalar.copy(kT[:, n * SS : (n + 1) * SS], psk[:])
                nc.vector.tensor_copy(vT[:], psv[:])
                for m in range(QH):
                    dst = qT[:, m * S + n * SS : m * S + (n + 1) * SS]
                    if m % 2 == 0:
                        nc.scalar.copy(dst, psq[m][:])
                    else:
                        nc.vector.tensor_copy(dst, psq[m][:])

                # V transpose into [s-within-tile, t*d] layout
                for t in range(4):
                    pst = ppt.tile([128, 128], F32, tag="pst")
                    nc.tensor.transpose(pst[:], vT[:, t * 128 : (t + 1) * 128], id_sb[:])
                    nc.scalar.copy(v_sb[:, (4 * n + t) * 128 : (4 * n + t + 1) * 128], pst[:])

                # RoPE on the 4 q slices + k slice of this stripe:
                # psr = P_rot @ src (holds [-x2; x1]); src = src*cos + psr*sin
                # (last stripe's rope is emitted inside phase 2 so the pool
                # barrier does not serialize behind its DVE chain)
                if n < NQS - 1:
                    csl = cos_sb[:, n * SS : (n + 1) * SS]
                    ssl = sin_sb[:, n * SS : (n + 1) * SS]
                    for i in range(QH + 1):
                        src = (
                            qT[:, i * S + n * SS : i * S + (n + 1) * SS]
                            if i < QH
                            else kT[:, n * SS : (n + 1) * SS]
                        )
                        psr = ppr.tile([128, SS], F32, tag="psr")
                        nc.tensor.matmul(psr[:], prot_sb[:], src, start=True, stop=True)
                        t2 = rsc.tile([128, SS], F32, tag="t2")
                        nc.vector.tensor_mul(t2[:], psr[:], ssl)
                        nc.vector.tensor_mul(src, src, csl)
                        nc.vector.tensor_add(src, src, t2[:])

        def emit_p23_preamble():
            # standalone timing: the inits phase 1 would have done
            gp.dma_start(prot_sb[:], protD[:, :].bitcast(FR))
            gp.dma_start(cos_sb[:], cosT[:, :])
            gp.dma_start(sin_sb[:], sinT[:, :])
            gp.dma_start(wo_all[:], woL[:, :])
            if causal:
                nc.sync.dma_start(md_sb[:], maskd[:, :].bitcast(FR))
            make_identity(nc, ones_f32[:])
            nc.vector.tensor_copy(id_fr[:], ones_f32[:])
            nc.vector.memset(ones_f32[:], 1.0)
            nc.vector.tensor_copy(ones_sb[:], ones_f32[:])
            nc.vector.memset(qT[:].bitcast(F32), 0.01)
            nc.vector.memset(kT[:].bitcast(F32), 0.01)
            nc.scalar.activation(v_sb[:], kT[:], ACTF.Copy)
            nc.vector.memset(ot[:], 0.01)

    # ---------------- phase 2+3: attention + output projection ----------
        def emit_phase2(rope_last: bool):
          with (
            tc.tile_pool(name="epool", bufs=8) as ep,
            tc.tile_pool(name="esump", bufs=2) as esp,
            tc.tile_pool(name="recp", bufs=2) as rp_,
            tc.tile_pool(name="mrowp", bufs=4) as mp,
            tc.tile_pool(name="yout", bufs=2) as yp,
            tc.tile_pool(name="pss", bufs=2, space="PSUM") as pps,
            tc.tile_pool(name="pso", bufs=1, space="PSUM") as ppo,
            tc.tile_pool(name="psd", bufs=1, space="PSUM") as ppd,
            tc.tile_pool(name="psy", bufs=2, space="PSUM") as ppy,
          ):
            def scores_block(qs, h):
                """S^T chunks -> mask -> exp; returns the live e chunks."""
                nkt = 4 * qs + 4 if causal else NKT
                qsl = qT[:, h * S + qs * SS : h * S + (qs + 1) * SS]
                es = []
                for c in range(nkt // 2):
                    pss = pps.tile([128, 2 * SS], F32, tag="pss")
                    if not causal:
                        mrow = mp.tile([128, 2 * SS], FR, tag="mrow")
                        for u in range(2):
                            t = 2 * c + u
                            nc.sync.dma_start(
                                mrow[:, u * SS : (u + 1) * SS],
                                maskT[
                                    t * 128 : (t + 1) * 128, qs * SS : (qs + 1) * SS
                                ].bitcast(FR),
                            )
                    for u in range(2):
                        t = 2 * c + u
                        masked = (causal and t >= 4 * qs) or not causal
                        nc.tensor.matmul(
                            pss[:, u * SS : (u + 1) * SS],
                            kT[:, t * 128 : (t + 1) * 128],
                            qsl,
                            start=True,
                            stop=not masked,
                        )
                        if masked:
                            # accumulate the additive mask on the PE via an
                            # identity-stationary matmul
                            mslice = (
                                md_sb[:, (t - 4 * qs) * SS : (t - 4 * qs + 1) * SS]
                                if causal
                                else mrow[:, u * SS : (u + 1) * SS]
                            )
                            nc.tensor.matmul(
                                pss[:, u * SS : (u + 1) * SS],
                                id_fr[:],
                                mslice,
                                start=False,
                                stop=True,
                            )
                    e = ep.tile([128, 2 * SS], FR, tag="e")
                    nc.scalar.activation(e[:], pss[:], ACTF.Exp)
                    es.append(e)
                return es

            def av_block(qs, h, es):
                nkt = 4 * qs + 4 if causal else NKT
                pso = ppo.tile([128, SS], F32, tag="pso")
                psd = ppd.tile([128, SS], F32, tag="psd")
                # denominator: sum the e chunks on DVE (the serial chain hides
                # under the exp latency), then a single ones-matmul pair does
                # the k-partition reduce broadcast to all 128 partitions
                esum = es[0]
                if len(es) > 1:
                    esum = esp.tile([128, 2 * SS], FR, tag="esum")
                    nc.vector.tensor_add(esum[:], es[0][:], es[1][:])
                    for e in es[2:]:
                        nc.vector.tensor_add(esum[:], esum[:], e[:])
                for c, e in enumerate(es):
                    for u in range(2):
                        t = 2 * c + u
                        er = e[:, u * SS : (u + 1) * SS]
                        nc.tensor.matmul(
                            pso[:],
                            v_sb[:, t * 128 : (t + 1) * 128],
                            er,
                            start=(t == 0),
                            stop=(t == nkt - 1),
                        )
                for u in range(2):
                    nc.tensor.matmul(
                        psd[:],
                        ones_sb[:],
                        esum[:, u * SS : (u + 1) * SS],
                        start=(u == 0),
                        stop=(u == 1),
                    )
                rec = rp_.tile([128, SS], FR, tag="rec")
                with nc.allow_low_precision(reason="fp32r recip feeds dve mul"):
                    nc.vector.reciprocal(rec[:], psd[:])
                od = ot[:, h * S + qs * SS : h * S + (qs + 1) * SS]
                nc.vector.tensor_mul(od, pso[:], rec[:])

            def proj_block(st):
                yt = yp.tile([128, HID], F32, tag="yt")
                for nn in range(HID // SS):
                    psy = ppy.tile([128, SS], F32, tag="psy")
                    for hh in range(QH):
                        nc.tensor.matmul(
                            psy[:],
                            ot[:, hh * S + st * 128 : hh * S + (st + 1) * 128],
                            wo_all[:, hh * HID + nn * SS : hh * HID + (nn + 1) * SS],
                            start=(hh == 0),
                            stop=(hh == QH - 1),
                        )
                    dst = yt[:, nn * SS : (nn + 1) * SS]
                    if nn % 2 == 0:
                        nc.scalar.copy(dst, psy[:])
                    else:
                        nc.vector.tensor_copy(dst, psy[:])
                    if nn == 3:
                        eng = nc.sync if (st % 2 == 0) else gp
                        eng.dma_start(
                            y[st * 128 : (st + 1) * 128, 0 : HID // 2],
                            yt[:, 0 : HID // 2],
                        )
                eng = gp if (st % 2 == 0) else nc.sync
                eng.dma_start(
                    y[st * 128 : (st + 1) * 128, HID // 2 :],
                    yt[:, HID // 2 :],
                )

            if rope_last:
                n3 = NQS - 1
                csl = cos_sb[:, n3 * SS : (n3 + 1) * SS]
                ssl = sin_sb[:, n3 * SS : (n3 + 1) * SS]
                for i in range(QH + 1):
                    src = (
                        qT[:, i * S + n3 * SS : i * S + (n3 + 1) * SS]
                        if i < QH
                        else kT[:, n3 * SS : (n3 + 1) * SS]
                    )
                    psr = ppy.tile([128, SS], F32, tag="psy")
                    nc.tensor.matmul(psr[:], prot_sb[:], src, start=True, stop=True)
                    t2 = rp_.tile([128, SS], FR, tag="rec")
                    nc.vector.tensor_mul(t2[:], psr[:], ssl)
                    nc.vector.tensor_mul(src, src, csl)
                    nc.vector.tensor_add(src, src, t2[:])

            # per head: scores/exp, then a proj piece of the previous stripe
            # (PE-dense filler while Act/DVE chase), then the AV accumulation
            for qs in range(NQS):
                for h in range(QH):
                    es = scores_block(qs, h)
                    if qs >= 1:
                        proj_block(4 * (qs - 1) + h)
                    av_block(qs, h, es)
            for st in range(4 * (NQS - 1), 4 * NQS):
                proj_block(st)

        if phases == "all":
            emit_phase1()
            emit_phase2(rope_last=True)
        elif phases == "p1":
            emit_phase1()
        elif phases == "p23":
            emit_p23_preamble()
            emit_phase2(rope_last=True)
        elif phases == "split":
            assert loop_n is not None
            with tc.For_i(0, loop_n, 1):
                emit_phase1()
            with tc.For_i(0, loop_n, 1):
                emit_phase2(rope_last=True)
        else:
            raise ValueError(phases)

    if split_waits:
        _split_multi_waits(nc)
    _BUILD_CACHE[key] = nc
    return nc


def _causal_mask_ref() -> np.ndarray:
    return np.triu(np.full((S, S), NEG, np.float32), k=1)


def _diag_mask_tiles() -> np.ndarray:
    p = np.arange(128, dtype=np.int64)[:, None]
    f = np.arange(SS, dtype=np.int64)[None, :]
    cols = [
        np.where(128 * j + p > f, np.float32(NEG), np.float32(0.0)) for j in range(4)
    ]
    return np.ascontiguousarray(np.concatenate(cols, axis=1).astype(np.float32))


def _prot() -> np.ndarray:
    """P with (P^T @ x)[d] = -x[d+64] for d<64, x[d-64] for d>=64."""
    P = np.zeros((128, 128), np.float32)
    for d in range(64):
        P[d + 64, d] = -1.0
        P[d, d + 64] = 1.0
    return P


def make_in_maps(hidden_states, attention_mask, cos, sin, wq, wk, wv, wo):
    """Host-side sharding/preprocessing. Returns (causal, in_maps)."""
    h = np.ascontiguousarray(np.asarray(hidden_states, dtype=np.float32)[0])
    m2 = np.ascontiguousarray(np.asarray(attention_mask, dtype=np.float32)[0, 0])
    wq = np.asarray(wq, dtype=np.float32)
    wk = np.asarray(wk, dtype=np.float32)
    wv = np.asarray(wv, dtype=np.float32)
    wo = np.asarray(wo, dtype=np.float32)

    causal = bool(np.array_equal(m2, _causal_mask_ref()))
    bf16 = mybir.dt.np(BF16)
    hT = np.ascontiguousarray(h.T).astype(bf16)
    cosT = np.ascontiguousarray(np.asarray(cos, dtype=np.float32)[0].T)
    sinT = np.ascontiguousarray(np.asarray(sin, dtype=np.float32)[0].T)
    prot = _prot()
    sc = np.float32(1.0 / math.sqrt(HD))
    if causal:
        md = _diag_mask_tiles()
    else:
        mT = np.ascontiguousarray(m2.T)

    in_maps = []
    for c in range(NCORES):
        wqT = (wq[c * QH * HD : (c + 1) * QH * HD] * sc).T  # [HID, 512]
        wkT = wk[c * HD : (c + 1) * HD].T                   # [HID, 128]
        wvT = wv[c * HD : (c + 1) * HD].T                   # [HID, 128]
        woT = wo[:, c * QH * HD : (c + 1) * QH * HD].T      # [512, HID]
        im = {
            "hT": hT,
            "cosT": cosT,
            "sinT": sinT,
            "protD": prot,
            # [128, KH*512]: wqL[p, k*512+j] = wqT[k*128+p, j]
            "wqL": np.ascontiguousarray(
                wqT.reshape(KH, 128, QH * HD).transpose(1, 0, 2).reshape(128, -1)
            ).astype(bf16),
            "wkL": np.ascontiguousarray(
                wkT.reshape(KH, 128, HD).transpose(1, 0, 2).reshape(128, -1)
            ).astype(bf16),
            "wvL": np.ascontiguousarray(
                wvT.reshape(KH, 128, HD).transpose(1, 0, 2).reshape(128, -1)
            ).astype(bf16),
            # [128, QH*HID]: woL[p, hh*HID+j] = woT[hh*128+p, j]
            "woL": np.ascontiguousarray(
                woT.reshape(QH, 128, HID).transpose(1, 0, 2).reshape(128, -1)
            ).astype(bf16),
        }
        if causal:
            im["maskd"] = md
        else:
            im["maskT"] = mT
        in_maps.append(im)
    return causal, in_maps


def kernel(hidden_states, attention_mask, cos, sin, wq, wk, wv, wo):
    causal, in_maps = make_in_maps(
        hidden_states, attention_mask, cos, sin, wq, wk, wv, wo
    )
    nc = _build(causal)
    res = run_bass_kernel_spmd(nc, in_maps, list(range(NCORES)))
    out = np.zeros((S, HID), np.float64)
    for c in range(NCORES):
        out += res.results[c]["y"].astype(np.float64)
    return out.reshape(B, S, HID).astype(np.float32)


# revision 4
# speedup vs baseline: 1.1418x; 1.1418x over previous
"""Trainium2 Bass kernel for MllamaTextSelfAttention (B=1, S=2048, HID=4096,
32 Q heads / 8 KV heads, HD=128, RoPE, causal mask, GQA).

Sharding: tensor-parallel over heads across 8 NeuronCores. Core c computes
Q heads [4c, 4c+4) and KV head c, plus the matching slice of the output
projection; the 8 partial outputs are summed on the host.

Dataflow (per core; fp32r matmuls except bf16 weights/h; free dim 512):
  phase 1 - batched bf16 weight DMAs in consumption order; ht chunks stream
    with deep buffering (issues on SP + Pool-SWDGE, or SP+Act in the For_i
    timing build); W-stationary QKV projections; RoPE as a rotate-matmul on
    the PE (psr = P_rot @ x) plus 3 DVE ops; V transposed via the PE.
  phase 2+3 - per (qs, h): S^T chunks [128,1024] in PSUM, causal diag mask
    accumulated on the PE via an identity-stationary matmul, exp on Act
    straight from PSUM; denominator = DVE chunk-sum + a single
    ones[128,128]-matmul pair (k-partition reduce + broadcast);
    od = pso * recip(psd) in one DVE op; the output projection is
    interleaved one stripe behind attention as PE-dense filler.
"""

import math
import os
import sys

for _p in (
    "/opt/trn_rl_repo",
    "/root/.axon_site",
    "/root/.axon_site/_ro/trn_rl_repo",
    "/root/.axon_site/_ro/pypackages",
):
    if os.path.isdir(_p) and _p not in sys.path:
        sys.path.append(_p)

import numpy as np
from contextlib import ExitStack

import concourse.bass as bass
import concourse.tile as tile
from concourse import mybir
from concourse.bass_utils import run_bass_kernel_spmd
from concourse.masks import make_identity

F32 = mybir.dt.float32
FR = mybir.dt.float32r
BF16 = mybir.dt.bfloat16
ACTF = mybir.ActivationFunctionType

B, S, HID = 1, 2048, 4096
NH, NKV, HD = 32, 8, 128
NCORES = 8
QH = NH // NCORES          # 4 q heads per core
SS = 512                   # stripe / matmul free dim
NQS = S // SS              # 4 stripes
NKT = S // 128             # 16 k tiles
KH = HID // 128            # 32 hidden-dim chunks
NEG = -1e9


def _split_multi_waits(nc: bass.Bass, only=None):
    """Walrus in this container encodes at most ONE sync-wait command per
    instruction. Hoist extra waits onto injected same-engine NoOps placed
    immediately before the instruction; engines are in-order so the
    semantics are unchanged. With only=("DMACopy","Drain"), compute
    instructions keep their multi-waits (fewer NoOps) -- valid only if the
    compiler accepts multi-wait on compute encodings."""
    n = 0
    for fn in nc.m.functions:
        for bb in fn.blocks:
            out = []
            for inst in bb.instructions:
                if only is not None and not any(k in type(inst).__name__ for k in only):
                    out.append(inst)
                    continue
                si = inst.sync_info
                if si is not None and si.on_wait and len(si.on_wait) > 1:
                    waits = list(si.on_wait)
                    for w in waits[:-1]:
                        n += 1
                        nop = mybir.InstNoOp(name=f"I-swait-{n}", ins=[], outs=[])
                        nop.engine = inst.engine
                        nop.sync_info = mybir.SyncInfo(on_wait=[w], on_update=[])
                        out.append(nop)
                    si.on_wait = [waits[-1]]
                out.append(inst)
            bb.instructions[:] = out
    return nc


_BUILD_CACHE = {}


def _build(causal: bool, split_waits: bool = True, loop_n=None, phases="all") -> bass.Bass:
    key = (causal, split_waits, loop_n, phases)
    if key in _BUILD_CACHE:
        return _BUILD_CACHE[key]

    nc = bass.Bass()
    hT = nc.dram_tensor("hT", [HID, S], BF16, kind="ExternalInput")
    wqL = nc.dram_tensor("wqL", [128, KH * 512], BF16, kind="ExternalInput")
    wkL = nc.dram_tensor("wkL", [128, KH * 128], BF16, kind="ExternalInput")
    wvL = nc.dram_tensor("wvL", [128, KH * 128], BF16, kind="ExternalInput")
    woL = nc.dram_tensor("woL", [128, QH * HID], BF16, kind="ExternalInput")
    cosT = nc.dram_tensor("cosT", [128, S], F32, kind="ExternalInput")
    sinT = nc.dram_tensor("sinT", [128, S], F32, kind="ExternalInput")
    protD = nc.dram_tensor("protD", [128, 128], F32, kind="ExternalInput")
    maskd = maskT = None
    if causal:
        maskd = nc.dram_tensor("maskd", [128, 4 * SS], F32, kind="ExternalInput")
    else:
        maskT = nc.dram_tensor("maskT", [S, S], F32, kind="ExternalInput")
    y = nc.dram_tensor("y", [S, HID], F32, kind="ExternalOutput")

    with tile.TileContext(nc) as tc, ExitStack() as ctx:
        # SWDGE (gpsimd) DMA inside a For_i fails this walrus' codegen; the
        # timing variants route the Pool-issued DMAs through SP instead.
        gp = nc.sync if loop_n is not None else nc.gpsimd

        if loop_n is not None and phases != "split":
            ctx.enter_context(tc.For_i(0, loop_n, 1))

        outer = ctx.enter_context(tc.tile_pool(name="outer", bufs=1))
        qT = outer.tile([128, QH * S], FR)      # [d, h*s] rope'd Q
        kT = outer.tile([128, S], FR)           # [d, s]   rope'd K
        v_sb = outer.tile([128, S], FR)         # [s-within-tile, t*d]
        ot = outer.tile([128, QH * S], BF16)    # [d, h*s] normalized O^T
        wo_all = outer.tile([128, QH * HID], BF16)
        ones_f32 = outer.tile([128, 128], F32)
        ones_sb = outer.tile([128, 128], FR)
        cos_sb = outer.tile([128, S], F32)
        sin_sb = outer.tile([128, S], F32)
        prot_sb = outer.tile([128, 128], FR)
        id_fr = outer.tile([128, 128], FR)
        md_sb = None
        if causal:
            md_sb = outer.tile([128, 4 * SS], FR, name="md_sb")

    # ---------------- phase 1: QKV + RoPE + V transpose -----------------
        def emit_phase1():
          with (
            tc.tile_pool(name="wqkv", bufs=1) as wp,
            tc.tile_pool(name="hstream", bufs=8) as hp,
            tc.tile_pool(name="rsc", bufs=1) as rsc,
            tc.tile_pool(name="ps1", bufs=1, space="PSUM") as pp1,
            tc.tile_pool(name="psr", bufs=1, space="PSUM") as ppr,
            tc.tile_pool(name="pst", bufs=1, space="PSUM") as ppt,
          ):
            wq_all = wp.tile([128, KH * 512], BF16)
            wk_all = wp.tile([128, KH * 128], BF16)
            wv_all = wp.tile([128, KH * 128], BF16)
            id_sb = wp.tile([128, 128], F32)
            vT = wp.tile([128, SS], F32)

            # batched weight loads, issued in chunk-consumption order so the
            # stripe-0 matmuls stream as parts arrive
            def wq_part(p):  # 4 chunks each
                gp.dma_start(wq_all[:, p * 2048 : (p + 1) * 2048],
                             wqL[:, p * 2048 : (p + 1) * 2048])
            def wkv_part(p):  # 8 chunks each
                gp.dma_start(wk_all[:, p * 1024 : (p + 1) * 1024],
                             wkL[:, p * 1024 : (p + 1) * 1024])
                gp.dma_start(wv_all[:, p * 1024 : (p + 1) * 1024],
                             wvL[:, p * 1024 : (p + 1) * 1024])
            wq_part(0); wkv_part(0); wq_part(1)
            wkv_part(1); wq_part(2); wq_part(3)
            wkv_part(2); wq_part(4); wq_part(5)
            wkv_part(3); wq_part(6); wq_part(7)
            gp.dma_start(prot_sb[:], protD[:, :].bitcast(FR))
            # after the weight parts: lands ~40us in, before stripe-0 rope
            gp.dma_start(cos_sb[:], cosT[:, :])
            gp.dma_start(sin_sb[:], sinT[:, :])
            make_identity(nc, id_sb[:])
            nc.vector.tensor_copy(id_fr[:], id_sb[:])
            nc.vector.memset(ones_f32[:], 1.0)
            nc.vector.tensor_copy(ones_sb[:], ones_f32[:])

            # ht streams on SP + Act; Pool still owns the weight-preamble
            # issues, so routing ht there would queue stripe-0 chunks behind
            # ~25us of weight issues
            dma_engs = [nc.sync, nc.scalar]
            for n in range(NQS):
                if n == 2:
                    nc.sync.dma_start(wo_all[:], woL[:, :])
                    if causal:
                        nc.sync.dma_start(md_sb[:], maskd[:, :].bitcast(FR))
                psq = [
                    pp1.tile([128, SS], F32, name=f"psq{m}", tag=f"psq{m}")
                    for m in range(QH)
                ]
                psk = pp1.tile([128, SS], F32, tag="psk")
                psv = pp1.tile([128, SS], F32, tag="psv")
                for k in range(KH):
                    ht = hp.tile([128, SS], BF16, tag="ht")
                    dma_engs[k % 2].dma_start(
                        ht[:],
                        hT[k * 128 : (k + 1) * 128, n * SS : (n + 1) * SS],
                    )
                    st, sp = (k == 0), (k == KH - 1)
                    for m in range(QH):
                        nc.tensor.matmul(
                            psq[m][:],
                            wq_all[:, k * 512 + m * 128 : k * 512 + (m + 1) * 128],
                            ht[:],
                            start=st,
                            stop=sp,
                        )
                    nc.tensor.matmul(
                        psk[:], wk_all[:, k * 128 : (k + 1) * 128], ht[:],
                        start=st, stop=sp,
                    )
                    nc.tensor.matmul(
                        psv[:], wv_all[:, k * 128 : (k + 1) * 128], ht[:],
                        start=st, stop=sp,
                    )
                # PSUM -> SBUF copies, split across Act and DVE so the next
                # stripe's start matmuls unblock quickly
                nc.scalar.copy(kT[:, n * SS : (n + 1) * SS], psk[:])
                nc.vector.tensor_copy(vT[:], psv[:])
                for m in range(QH):
                    dst = qT[:, m * S + n * SS : m * S + (n + 1) * SS]
                    if m % 2 == 0:
                        nc.scalar.copy(dst, psq[m][:])
                    else:
                        nc.vector.tensor_copy(dst, psq[m][:])

                # V transpose into [s-within-tile, t*d] layout
                for t in range(4):
                    pst = ppt.tile([128, 128], F32, tag="pst")
                    nc.tensor.transpose(pst[:], vT[:, t * 128 : (t + 1) * 128], id_sb[:])
                    nc.scalar.copy(v_sb[:, (4 * n + t) * 128 : (4 * n + t + 1) * 128], pst[:])

                # RoPE on the 4 q slices + k slice of this stripe:
                # psr = P_rot @ src (holds [-x2; x1]); src = src*cos + psr*sin
                # (last stripe's rope is emitted inside phase 2 so the pool
                # barrier does not serialize behind its DVE chain)
                if n < NQS - 1:
                    csl = cos_sb[:, n * SS : (n + 1) * SS]
                    ssl = sin_sb[:, n * SS : (n + 1) * SS]
                    for i in range(QH + 1):
                        src = (
                            qT[:, i * S + n * SS : i * S + (n + 1) * SS]
                            if i < QH
                            else kT[:, n * SS : (n + 1) * SS]
                        )
                        psr = ppr.tile([128, SS], F32, tag="psr")
                        nc.tensor.matmul(psr[:], prot_sb[:], src, start=True, stop=True)
                        t2 = rsc.tile([128, SS], F32, tag="t2")
                        nc.vector.tensor_mul(t2[:], psr[:], ssl)
                        nc.vector.tensor_mul(src, src, csl)
                        nc.vector.tensor_add(src, src, t2[:])

        def emit_p23_preamble():
            # standalone timing: the inits phase 1 would have done
            gp.dma_start(prot_sb[:], protD[:, :].bitcast(FR))
            gp.dma_start(cos_sb[:], cosT[:, :])
            gp.dma_start(sin_sb[:], sinT[:, :])
            gp.dma_start(wo_all[:], woL[:, :])
            if causal:
                nc.sync.dma_start(md_sb[:], maskd[:, :].bitcast(FR))
            make_identity(nc, ones_f32[:])
            nc.vector.tensor_copy(id_fr[:], ones_f32[:])
            nc.vector.memset(ones_f32[:], 1.0)
            nc.vector.tensor_copy(ones_sb[:], ones_f32[:])
            nc.vector.memset(qT[:].bitcast(F32), 0.01)
            nc.vector.memset(kT[:].bitcast(F32), 0.01)
            nc.scalar.activation(v_sb[:], kT[:], ACTF.Copy)
            nc.vector.memset(ot[:], 0.01)

    # ---------------- phase 2+3: attention + output projection ----------
        def emit_phase2(rope_last: bool):
          with (
            tc.tile_pool(name="epool", bufs=8) as ep,
            tc.tile_pool(name="esump", bufs=2) as esp,
            tc.tile_pool(name="recp", bufs=2) as rp_,
            tc.tile_pool(name="mrowp", bufs=4) as mp,
            tc.tile_pool(name="yout", bufs=2) as yp,
            tc.tile_pool(name="pss", bufs=2, space="PSUM") as pps,
            tc.tile_pool(name="pso", bufs=1, space="PSUM") as ppo,
            tc.tile_pool(name="psd", bufs=1, space="PSUM") as ppd,
            tc.tile_pool(name="psy", bufs=2, space="PSUM") as ppy,
          ):
            def scores_block(qs, h):
                """S^T chunks -> mask -> exp; returns the live e chunks."""
                nkt = 4 * qs + 4 if causal else NKT
                qsl = qT[:, h * S + qs * SS : h * S + (qs + 1) * SS]
                es = []
                for c in range(nkt // 2):
                    pss = pps.tile([128, 2 * SS], F32, tag="pss")
                    if not causal:
                        mrow = mp.tile([128, 2 * SS], FR, tag="mrow")
                        for u in range(2):
                            t = 2 * c + u
                            nc.sync.dma_start(
                                mrow[:, u * SS : (u + 1) * SS],
                                maskT[
                                    t * 128 : (t + 1) * 128, qs * SS : (qs + 1) * SS
                                ].bitcast(FR),
                            )
                    for u in range(2):
                        t = 2 * c + u
                        masked = (causal and t >= 4 * qs) or not causal
                        nc.tensor.matmul(
                            pss[:, u * SS : (u + 1) * SS],
                            kT[:, t * 128 : (t + 1) * 128],
                            qsl,
                            start=True,
                            stop=not masked,
                        )
                        if masked:
                            # accumulate the additive mask on the PE via an
                            # identity-stationary matmul
                            mslice = (
                                md_sb[:, (t - 4 * qs) * SS : (t - 4 * qs + 1) * SS]
                                if causal
                                else mrow[:, u * SS : (u + 1) * SS]
                            )
                            nc.tensor.matmul(
                                pss[:, u * SS : (u + 1) * SS],
                                id_fr[:],
                                mslice,
                                start=False,
                                stop=True,
                            )
                    e = ep.tile([128, 2 * SS], FR, tag="e")
                    nc.scalar.activation(e[:], pss[:], ACTF.Exp)
                    es.append(e)
                return es

            def av_block(qs, h, es):
                nkt = 4 * qs + 4 if causal else NKT
                pso = ppo.tile([128, SS], F32, tag="pso")
                psd = ppd.tile([128, SS], F32, tag="psd")
                # denominator: sum the e chunks on DVE (the serial chain hides
                # under the exp latency), then a single ones-matmul pair does
                # the k-partition reduce broadcast to all 128 partitions
                esum = es[0]
                if len(es) > 1:
                    esum = esp.tile([128, 2 * SS], FR, tag="esum")
                    nc.vector.tensor_add(esum[:], es[0][:], es[1][:])
                    for e in es[2:]:
                        nc.vector.tensor_add(esum[:], esum[:], e[:])
                for c, e in enumerate(es):
                    for u in range(2):
                        t = 2 * c + u
                        er = e[:, u * SS : (u + 1) * SS]
                        nc.tensor.matmul(
                            pso[:],
                            v_sb[:, t * 128 : (t + 1) * 128],
                            er,
                            start=(t == 0),
                            stop=(t == nkt - 1),
                        )
                for u in range(2):
                    nc.tensor.matmul(
                        psd[:],
                        ones_sb[:],
                        esum[:, u * SS : (u + 1) * SS],
                        start=(u == 0),
                        stop=(u == 1),
                    )
                rec = rp_.tile([128, SS], FR, tag="rec")
                with nc.allow_low_precision(reason="fp32r recip feeds dve mul"):
                    nc.vector.reciprocal(rec[:], psd[:])
                od = ot[:, h * S + qs * SS : h * S + (qs + 1) * SS]
                nc.vector.tensor_mul(od, pso[:], rec[:])

            def proj_block(st):
                yt = yp.tile([128, HID], F32, tag="yt")
                for nn in range(HID // SS):
                    psy = ppy.tile([128, SS], F32, tag="psy")
                    for hh in range(QH):
                        nc.tensor.matmul(
                            psy[:],
                            ot[:, hh * S + st * 128 : hh * S + (st + 1) * 128],
                            wo_all[:, hh * HID + nn * SS : hh * HID + (nn + 1) * SS],
                            start=(hh == 0),
                            stop=(hh == QH - 1),
                        )
                    dst = yt[:, nn * SS : (nn + 1) * SS]
                    if nn % 2 == 0:
                        nc.scalar.copy(dst, psy[:])
                    else:
                        nc.vector.tensor_copy(dst, psy[:])
                    if nn == 3:
                        eng = nc.sync if (st % 2 == 0) else gp
                        eng.dma_start(
                            y[st * 128 : (st + 1) * 128, 0 : HID // 2],
                            yt[:, 0 : HID // 2],
                        )
                eng = gp if (st % 2 == 0) else nc.sync
                eng.dma_start(
                    y[st * 128 : (st + 1) * 128, HID // 2 :],
                    yt[:, HID // 2 :],
                )

            if rope_last:
                n3 = NQS - 1
                csl = cos_sb[:, n3 * SS : (n3 + 1) * SS]
                ssl = sin_sb[:, n3 * SS : (n3 + 1) * SS]
                for i in range(QH + 1):
                    src = (
                        qT[:, i * S + n3 * SS : i * S + (n3 + 1) * SS]
                        if i < QH
                        else kT[:, n3 * SS : (n3 + 1) * SS]
                    )
                    psr = ppy.tile([128, SS], F32, tag="psy")
                    nc.tensor.matmul(psr[:], prot_sb[:], src, start=True, stop=True)
                    t2 = rp_.tile([128, SS], FR, tag="rec")
                    nc.vector.tensor_mul(t2[:], psr[:], ssl)
                    nc.vector.tensor_mul(src, src, csl)
                    nc.vector.tensor_add(src, src, t2[:])

            # per head: scores/exp, then a proj piece of the previous stripe
            # (PE-dense filler while Act/DVE chase), then the AV accumulation
            for qs in range(NQS):
                for h in range(QH):
                    es = scores_block(qs, h)
                    if qs >= 1:
                        proj_block(4 * (qs - 1) + h)
                    av_block(qs, h, es)
            for st in range(4 * (NQS - 1), 4 * NQS):
                proj_block(st)

        if phases == "all":
            emit_phase1()
            emit_phase2(rope_last=True)
        elif phases == "p1":
            emit_phase1()
        elif phases == "p23":
            emit_p23_preamble()
            emit_phase2(rope_last=True)
        elif phases == "split":
            assert loop_n is not None
            with tc.For_i(0, loop_n, 1):
                emit_phase1()
            with tc.For_i(0, loop_n, 1):
                emit_phase2(rope_last=True)
        else:
            raise ValueError(phases)

    if split_waits == "dma":
        _split_multi_waits(nc, only=("DMACopy", "Drain"))
    elif split_waits:
        _split_multi_waits(nc)
    _BUILD_CACHE[key] = nc
    return nc


def _causal_mask_ref() -> np.ndarray:
    return np.triu(np.full((S, S), NEG, np.float32), k=1)


def _diag_mask_tiles() -> np.ndarray:
    p = np.arange(128, dtype=np.int64)[:, None]
    f = np.arange(SS, dtype=np.int64)[None, :]
    cols = [
        np.where(128 * j + p > f, np.float32(NEG), np.float32(0.0)) for j in range(4)
    ]
    return np.ascontiguousarray(np.concatenate(cols, axis=1).astype(np.float32))


def _prot() -> np.ndarray:
    """P with (P^T @ x)[d] = -x[d+64] for d<64, x[d-64] for d>=64."""
    P = np.zeros((128, 128), np.float32)
    for d in range(64):
        P[d + 64, d] = -1.0
        P[d, d + 64] = 1.0
    return P


def make_in_maps(hidden_states, attention_mask, cos, sin, wq, wk, wv, wo):
    """Host-side sharding/preprocessing. Returns (causal, in_maps)."""
    h = np.ascontiguousarray(np.asarray(hidden_states, dtype=np.float32)[0])
    m2 = np.ascontiguousarray(np.asarray(attention_mask, dtype=np.float32)[0, 0])
    wq = np.asarray(wq, dtype=np.float32)
    wk = np.asarray(wk, dtype=np.float32)
    wv = np.asarray(wv, dtype=np.float32)
    wo = np.asarray(wo, dtype=np.float32)

    causal = bool(np.array_equal(m2, _causal_mask_ref()))
    bf16 = mybir.dt.np(BF16)
    hT = np.ascontiguousarray(h.T).astype(bf16)
    cosT = np.ascontiguousarray(np.asarray(cos, dtype=np.float32)[0].T)
    sinT = np.ascontiguousarray(np.asarray(sin, dtype=np.float32)[0].T)
    prot = _prot()
    sc = np.float32(1.0 / math.sqrt(HD))
    if causal:
        md = _diag_mask_tiles()
    else:
        mT = np.ascontiguousarray(m2.T)

    in_maps = []
    for c in range(NCORES):
        wqT = (wq[c * QH * HD : (c + 1) * QH * HD] * sc).T  # [HID, 512]
        wkT = wk[c * HD : (c + 1) * HD].T                   # [HID, 128]
        wvT = wv[c * HD : (c + 1) * HD].T                   # [HID, 128]
        woT = wo[:, c * QH * HD : (c + 1) * QH * HD].T      # [512, HID]
        im = {
            "hT": hT,
            "cosT": cosT,
            "sinT": sinT,
            "protD": prot,
            # [128, KH*512]: wqL[p, k*512+j] = wqT[k*128+p, j]
            "wqL": np.ascontiguousarray(
                wqT.reshape(KH, 128, QH * HD).transpose(1, 0, 2).reshape(128, -1)
            ).astype(bf16),
            "wkL": np.ascontiguousarray(
                wkT.reshape(KH, 128, HD).transpose(1, 0, 2).reshape(128, -1)
            ).astype(bf16),
            "wvL": np.ascontiguousarray(
                wvT.reshape(KH, 128, HD).transpose(1, 0, 2).reshape(128, -1)
            ).astype(bf16),
            # [128, QH*HID]: woL[p, hh*HID+j] = woT[hh*128+p, j]
            "woL": np.ascontiguousarray(
                woT.reshape(QH, 128, HID).transpose(1, 0, 2).reshape(128, -1)
            ).astype(bf16),
        }
        if causal:
            im["maskd"] = md
        else:
            im["maskT"] = mT
        in_maps.append(im)
    return causal, in_maps


def kernel(hidden_states, attention_mask, cos, sin, wq, wk, wv, wo):
    causal, in_maps = make_in_maps(
        hidden_states, attention_mask, cos, sin, wq, wk, wv, wo
    )
    nc = _build(causal)
    res = run_bass_kernel_spmd(nc, in_maps, list(range(NCORES)))
    out = np.zeros((S, HID), np.float64)
    for c in range(NCORES):
        out += res.results[c]["y"].astype(np.float64)
    return out.reshape(B, S, HID).astype(np.float32)
